# revision 32
# baseline (speedup 1.0000x reference)
"""Gemma3-style sliding-window attention on 8 Trainium2 NeuronCores.

Sharding: tensor-parallel over the 8 query heads (1 head per core, KV head
h//2 replicated per pair). Each core computes its head's partial o-proj
output [S, HID]; the host sums the 8 partials.

All matmul operands are bf16 (rel-err budget 2e-2); accumulation is f32 in
PSUM. Everything the device consumes transposed is pre-transposed on the
host, so the device issues only natural-layout matmuls.

Softmax trick: scores are softcapped by 50*tanh(.), so they are bounded in
[-50, 50] and exp() never overflows f32 -> no running-max subtraction.
Masking is additive (-2000) on the tanh output (pre-exp), which makes
masked exp() terms exactly 0.
"""

import os
import sys
import types

import numpy as np
import ml_dtypes

BF16 = ml_dtypes.bfloat16

B, S, HID = 1, 2048, 2560
H, KV, D = 8, 4, 256
SCALE = 256 ** -0.5
SOFTCAP = 50.0
WINDOW = 512
EPS = 1e-6
NCORES = 8
DH = 2                 # 128-partition halves of D
NHID = HID // 128      # 20
SB = 512               # s-block size
NSB = S // SB          # 4
NBLK = S // 128        # 16 query blocks
MAXW = WINDOW + 128    # max key span per query block

_COMPILED = None
LAST_RESULT = None     # BassKernelResults of the most recent run (for test.py)


def _install_ntff_shim():
    """The image's antenv lacks axon_hooks; recreate it so trace=True works."""
    try:
        from antenv import axon_hooks  # noqa: F401
        return
    except ImportError:
        pass
    try:
        import antenv
        import trn_agent_boot.trn_boot as tb

        hook = tb._ntff_profile_via_ctypes("/opt/axon/libaxon_pjrt.so")
        mod = types.ModuleType("antenv.axon_hooks")
        mod._hook = hook
        mod.get_axon_ntff_profile_hook = lambda: mod._hook
        mod.set_axon_ntff_profile_hook = lambda h: setattr(mod, "_hook", h)
        sys.modules["antenv.axon_hooks"] = mod
        antenv.axon_hooks = mod
    except Exception:
        pass


def _build():
    import concourse.mybir as mybir
    import concourse.tile as tile
    from concourse import bacc
    from concourse.mybir import ActivationFunctionType as AF

    f32 = mybir.dt.float32
    bf16 = mybir.dt.bfloat16

    nc = bacc.Bacc("TRN2", target_bir_lowering=False, debug=False,
                   num_devices=NCORES)

    hsT_d = nc.dram_tensor("hsT", [128, NHID * S], bf16, kind="ExternalInput")
    wqT_d = nc.dram_tensor("wqT", [128, NHID * D], bf16, kind="ExternalInput")
    wkT_d = nc.dram_tensor("wkT", [128, NHID * D], bf16, kind="ExternalInput")
    wvT_d = nc.dram_tensor("wvT", [128, NHID * D], bf16, kind="ExternalInput")
    woT_d = nc.dram_tensor("woT", [D, HID], bf16, kind="ExternalInput")
    cosT_d = nc.dram_tensor("cosT", [D, S], bf16, kind="ExternalInput")
    sinT_d = nc.dram_tensor("sinT", [D, S], bf16, kind="ExternalInput")
    wtq_d = nc.dram_tensor("wtq", [D, 1], f32, kind="ExternalInput")
    wtk_d = nc.dram_tensor("wtk", [D, 1], f32, kind="ExternalInput")
    mask_d = nc.dram_tensor("maskadd", [128, MAXW], f32, kind="ExternalInput")
    id_d = nc.dram_tensor("ident", [128, 128], bf16, kind="ExternalInput")
    ones_d = nc.dram_tensor("ones_", [128, 1], bf16, kind="ExternalInput")
    out_d = nc.dram_tensor("out", [S, HID], bf16, kind="ExternalOutput")

    C2 = (SOFTCAP / SCALE) ** 2  # folds SCALE/SOFTCAP into the k-norm scale

    with tile.TileContext(nc) as tc:
        with tc.tile_pool(name="const", bufs=1) as cp, \
             tc.tile_pool(name="hstp", bufs=2) as hstp, \
             tc.tile_pool(name="evp", bufs=3) as evp, \
             tc.tile_pool(name="smp", bufs=2) as smp, \
             tc.tile_pool(name="psA", bufs=2, space="PSUM") as psA, \
             tc.tile_pool(name="psS1", bufs=2, space="PSUM") as psS1, \
             tc.tile_pool(name="psS2", bufs=1, space="PSUM") as psS2, \
             tc.tile_pool(name="psT", bufs=1, space="PSUM") as psT, \
             tc.tile_pool(name="psP", bufs=2, space="PSUM") as psP:

            # ---- persistent constants ----
            wq_sb = cp.tile([128, NHID * D], bf16, tag="wq", name="wq")
            wk_sb = cp.tile([128, NHID * D], bf16, tag="wk", name="wk")
            wv_sb = cp.tile([128, NHID * D], bf16, tag="wv", name="wv")
            wo_sb = [cp.tile([128, HID], bf16, tag=f"wo{d}", name=f"wo{d}")
                     for d in range(DH)]
            cos_sb = [cp.tile([128, S], bf16, tag=f"cos{d}", name=f"cos{d}")
                      for d in range(DH)]
            sin_sb = [cp.tile([128, S], bf16, tag=f"sin{d}", name=f"sin{d}")
                      for d in range(DH)]
            wtq_sb = [cp.tile([128, 1], f32, tag=f"wtq{d}", name=f"wtq{d}")
                      for d in range(DH)]
            wtk_sb = [cp.tile([128, 1], f32, tag=f"wtk{d}", name=f"wtk{d}")
                      for d in range(DH)]
            mask_sb = cp.tile([128, MAXW], f32, tag="mask", name="mask")
            id_sb = cp.tile([128, 128], bf16, tag="ident", name="ident")
            ones_sb = cp.tile([128, 1], bf16, tag="ones", name="ones")
            epsq_sb = cp.tile([128, 1], f32, tag="epsq", name="epsq")
            epsk_sb = cp.tile([128, 1], f32, tag="epsk", name="epsk")
            nc.vector.memset(epsq_sb, EPS)
            nc.vector.memset(epsk_sb, C2 * EPS)

            # persistent activations
            qwT = [cp.tile([128, S], bf16, tag=f"qwT{d}", name=f"qwT{d}")
                   for d in range(DH)]
            kwT = [cp.tile([128, S], bf16, tag=f"kwT{d}", name=f"kwT{d}")
                   for d in range(DH)]
            outT = [cp.tile([128, S], bf16, tag=f"outT{d}", name=f"outT{d}")
                    for d in range(DH)]
            v_sb = [cp.tile([128, D], bf16, tag=f"v{m}", name=f"v{m}")
                    for m in range(NBLK)]
            beta_bc = cp.tile([128, S], f32, tag="betabc", name="betabc")
            alpha = cp.tile([128, NBLK], f32, tag="alpha", name="alpha")
            dn = cp.tile([128, NBLK], f32, tag="dn", name="dn")
            rc = cp.tile([128, NBLK], f32, tag="rc", name="rc")

            # piecewise wide-line loads, interleaved so the first
            # projection matmuls start after the first piece lands
            SBW = NHID * SB      # columns per s-block in the streamed layout
            hst0 = hstp.tile([128, SBW], bf16, tag="hstbig", name="hst_sb0")
            NP = 10
            CP = NHID // NP      # weight chunks per piece
            for p in range(NP):
                wsl = slice(p * CP * D, (p + 1) * CP * D)
                nc.sync.dma_start(wq_sb[:, wsl], wqT_d.ap()[:, wsl])
                nc.sync.dma_start(wk_sb[:, wsl], wkT_d.ap()[:, wsl])
                hsl = slice(p * CP * SB, (p + 1) * CP * SB)
                nc.sync.dma_start(hst0[:, hsl], hsT_d.ap()[:, hsl])
            nc.sync.dma_start(wv_sb, wvT_d.ap())
            # needed from RoPE / softmax onward — after the first projection
            for d in range(DH):
                r = slice(d * 128, (d + 1) * 128)
                nc.sync.dma_start(cos_sb[d], cosT_d.ap()[r, :])
                nc.sync.dma_start(sin_sb[d], sinT_d.ap()[r, :])
                nc.sync.dma_start(wtq_sb[d], wtq_d.ap()[r, :])
                nc.sync.dma_start(wtk_sb[d], wtk_d.ap()[r, :])
            nc.sync.dma_start(mask_sb, mask_d.ap())
            nc.sync.dma_start(id_sb, id_d.ap())
            nc.sync.dma_start(ones_sb, ones_d.ap())
            # needed only at o-proj
            for d in range(DH):
                r = slice(d * 128, (d + 1) * 128)
                nc.sync.dma_start(wo_sb[d], woT_d.ap()[r, :])

            def emit_oproj_block(b):
                    qsl = slice(b * 128, (b + 1) * 128)
                    orow = smp.tile([128, HID], bf16, tag="orow", bufs=2,
                                    name=f"orow{b}")
                    for n in range(5):
                        op = psA.tile([128, SB], f32, tag="acc",
                                      name=f"op{b}_{n}")
                        nsl = slice(n * 512, (n + 1) * 512)
                        for d in range(DH):
                            nc.tensor.matmul(op, outT[d][:, qsl],
                                             wo_sb[d][:, nsl],
                                             start=(d == 0),
                                             stop=(d == DH - 1))
                        if n % 2 == 0:
                            nc.scalar.mul(orow[:, nsl], op, rc[:, b:b + 1])
                        else:
                            nc.vector.tensor_scalar_mul(orow[:, nsl], op,
                                                        rc[:, b:b + 1])
                    nc.sync.dma_start(out_d.ap()[b * 128:(b + 1) * 128, :],
                                      orow)

            hst_tiles = []
            for sb in range(NSB):
                s0 = sb * SB
                sl = slice(s0, s0 + SB)

                # ---- hidden-state stream for this s-block ----
                if sb == 0:
                    hst_tiles.append(hst0)
                if sb + 1 < NSB:
                    nxt = hstp.tile([128, SBW], bf16, tag="hstbig",
                                    name=f"hst_sb{sb + 1}")
                    nc.sync.dma_start(
                        nxt, hsT_d.ap()[:, (sb + 1) * SBW:(sb + 2) * SBW])
                    hst_tiles.append(nxt)
                hstb = hst_tiles[sb]

                # ---- q/k projections (transposed layout [d, s]) ----
                qsq = [evp.tile([128, SB], bf16, tag=f"qsq{d}",
                                name=f"qsq{sb}_{d}") for d in range(DH)]
                ksq = [evp.tile([128, SB], bf16, tag=f"ksq{d}",
                                name=f"ksq{sb}_{d}") for d in range(DH)]
                qtl = [evp.tile([128, SB], bf16, tag=f"qtl{d}",
                                name=f"qtl{sb}_{d}") for d in range(DH)]
                ktl = [evp.tile([128, SB], bf16, tag=f"ktl{d}",
                                name=f"ktl{sb}_{d}") for d in range(DH)]
                for wst, wtt, sqt, qt in ((wq_sb, wtq_sb, qsq, qtl),
                                          (wk_sb, wtk_sb, ksq, ktl)):
                    for d in range(DH):
                        pp = psA.tile([128, SB], f32, tag="acc",
                                      name=f"pp{sb}_{d}")
                        for t in range(NHID):
                            lsl = slice(t * D + d * 128,
                                        t * D + (d + 1) * 128)
                            nc.tensor.matmul(pp, wst[:, lsl],
                                             hstb[:, t * SB:(t + 1) * SB],
                                             start=(t == 0),
                                             stop=(t == NHID - 1))
                        nc.scalar.activation(sqt[d], pp, AF.Square)
                        nc.vector.tensor_scalar_mul(qt[d], pp, wtt[d])

                # ---- v projection (natural layout [s, d]) ----
                # emitted before RoPE: gives the PE matmul work while the
                # DVE runs the rope elementwise chain
                for m in range(4):
                    vp = psP.tile([128, D], f32, tag="pp",
                                   name=f"vp{sb}_{m}")
                    for t in range(NHID):
                        msl = slice(t * SB + m * 128, t * SB + (m + 1) * 128)
                        nc.tensor.matmul(vp, hstb[:, msl],
                                         wv_sb[:, t * D:(t + 1) * D],
                                         start=(t == 0), stop=(t == NHID - 1))
                    nc.vector.tensor_copy(v_sb[sb * 4 + m], vp)

                # ---- RoPE into persistent qwT/kwT (k first) ----
                for qt, dstT in ((ktl, kwT), (qtl, qwT)):
                    ra = smp.tile([128, SB], bf16, tag="ra", name=f"ra{sb}")
                    rb = smp.tile([128, SB], bf16, tag="rb", name=f"rb{sb}")
                    nc.vector.tensor_mul(ra, qt[0], cos_sb[0][:, sl])
                    nc.vector.tensor_mul(rb, qt[1], sin_sb[0][:, sl])
                    nc.vector.tensor_sub(dstT[0][:, sl], ra, rb)
                    rc2 = smp.tile([128, SB], bf16, tag="ra", name=f"rc{sb}")
                    rd = smp.tile([128, SB], bf16, tag="rb", name=f"rd{sb}")
                    nc.vector.tensor_mul(rc2, qt[1], cos_sb[1][:, sl])
                    nc.vector.tensor_mul(rd, qt[0], sin_sb[1][:, sl])
                    nc.vector.tensor_add(dstT[1][:, sl], rc2, rd)

    
                # previous s-block's o-proj: PE filler during rope/stats
                if sb > 0:
                    for mm in range(4):
                        emit_oproj_block((sb - 1) * 4 + mm)

                # ---- alpha = 1/sqrt(mean(q^2)+eps), per query row ----
                ap_ps = psP.tile([128, 4], f32, tag="pp",
                                 name=f"aps{sb}")
                for m in range(4):
                    msl = slice(m * 128, (m + 1) * 128)
                    for d in range(DH):
                        nc.tensor.matmul(ap_ps[:, m:m + 1], qsq[d][:, msl],
                                         ones_sb, start=(d == 0),
                                         stop=(d == DH - 1))
                atmp = smp.tile([128, 4], f32, tag="atmp", name=f"atmp{sb}")
                nc.scalar.activation(atmp, ap_ps, AF.Sqrt,
                                     bias=epsq_sb, scale=1.0 / D)
                nc.vector.reciprocal(alpha[:, sb * 4:(sb + 1) * 4], atmp)

                # ---- beta row = (SCALE/50)/sqrt(mean(k^2)+eps), bcast ----
                bp = psA.tile([1, SB], f32, tag="acc", name=f"bp{sb}")
                for d in range(DH):
                    nc.tensor.matmul(bp, ones_sb, ksq[d],
                                     start=(d == 0), stop=(d == DH - 1))
                btmp = smp.tile([1, SB], f32, tag="btmp", name=f"btmp{sb}")
                nc.scalar.activation(btmp, bp, AF.Sqrt,
                                     bias=epsk_sb[0:1, :], scale=C2 / D)
                brow = smp.tile([1, SB], f32, tag="brow", name=f"brow{sb}")
                nc.vector.reciprocal(brow, btmp)
                nc.gpsimd.partition_broadcast(beta_bc[:, sl], brow)

                # ---- attention + o-proj per 128-row query block ----
                for m in range(4):
                    b = sb * 4 + m
                    q0 = b * 128
                    w = min(b + 1, 5) * 128
                    k0 = q0 + 128 - w
                    w1 = w - 128
                    qsl = slice(q0, q0 + 128)

                    if w1 > 0:
                        sc1 = psS1.tile([128, SB], f32, tag="sc1",
                                        name=f"sc1_{b}")
                        for d in range(DH):
                            nc.tensor.matmul(sc1[:, 0:w1], qwT[d][:, qsl],
                                             kwT[d][:, k0:k0 + w1],
                                             start=(d == 0),
                                             stop=(d == DH - 1))
                    sc2 = psS2.tile([128, 128], f32, tag="sc2",
                                     name=f"sc2_{b}")
                    for d in range(DH):
                        nc.tensor.matmul(sc2, qwT[d][:, qsl], kwT[d][:, qsl],
                                         start=(d == 0), stop=(d == DH - 1))

                    traw = smp.tile([128, MAXW], f32, tag="traw", bufs=2,
                                    name=f"traw{b}")
                    if w1 > 0:
                        nc.vector.tensor_mul(traw[:, 0:w1], sc1[:, 0:w1],
                                             beta_bc[:, k0:k0 + w1])
                    nc.vector.tensor_mul(traw[:, w1:w], sc2, beta_bc[:, qsl])

                    tt = smp.tile([128, MAXW], f32, tag="tanh", bufs=2,
                                  name=f"tt{b}")
                    nc.scalar.activation(tt[:, 0:w], traw[:, 0:w], AF.Tanh,
                                         scale=alpha[:, b:b + 1])
                    nc.vector.tensor_add(tt[:, w1:w], tt[:, w1:w],
                                         mask_sb[:, 512:640])
                    if w == MAXW:
                        nc.vector.tensor_add(tt[:, 0:128], tt[:, 0:128],
                                             mask_sb[:, 0:128])

                    et = smp.tile([128, MAXW], bf16, tag="et", bufs=3,
                                  name=f"et{b}")
                    nc.scalar.activation(et[:, 0:w], tt[:, 0:w], AF.Exp,
                                         scale=SOFTCAP,
                                         accum_out=dn[:, b:b + 1])
                    # 1/denominator is applied at the o-proj eviction (rows
                    # of that PSUM are queries), keeping it off the softmax
                    # critical path
                    nc.vector.reciprocal(rc[:, b:b + 1], dn[:, b:b + 1])

                    nchunks = w // 128
                    etcs = []
                    for c in range(nchunks):
                        tp = psT.tile([128, 128], bf16, tag="tp",
                                      name=f"tp{b}_{c}")
                        nc.tensor.transpose(tp, et[:, c * 128:(c + 1) * 128],
                                            id_sb)
                        etc = smp.tile([128, 128], bf16, tag="etc", bufs=6,
                                       name=f"etc{b}_{c}")
                        nc.vector.tensor_copy(etc, tp)
                        etcs.append(etc)
                    po = psP.tile([128, D], f32, tag="pp", name=f"po{b}")
                    for d in range(DH):
                        dsl = slice(d * 128, (d + 1) * 128)
                        for c in range(nchunks):
                            kvi = k0 // 128 + c
                            nc.tensor.matmul(po[:, dsl], v_sb[kvi][:, dsl],
                                             etcs[c], start=(c == 0),
                                             stop=(c == nchunks - 1))
                    for d in range(DH):
                        dsl = slice(d * 128, (d + 1) * 128)
                        nc.vector.tensor_copy(outT[d][:, qsl], po[:, dsl])

                    if sb == NSB - 1:
                        emit_oproj_block(b)

    nc.compile()
    return nc


def _prep_in_maps(hidden_states, position_ids, cos_table, sin_table,
                  Wq, Wk, Wv, Wo, q_norm_w, k_norm_w):
    hs = np.asarray(hidden_states, np.float32).reshape(S, HID)
    pos = np.asarray(position_ids).reshape(S).astype(np.int64)
    cos = np.asarray(cos_table, np.float32)[pos]   # [S, D]
    sin = np.asarray(sin_table, np.float32)[pos]
    Wq = np.asarray(Wq, np.float32)
    Wk = np.asarray(Wk, np.float32)
    Wv = np.asarray(Wv, np.float32)
    Wo = np.asarray(Wo, np.float32)

    # streamed layout: [128, sb*(NHID*SB) + t*SB + s'] so every DMA line is
    # wide and contiguous
    hsT = np.ascontiguousarray(
        hs.T.astype(BF16).reshape(NHID, 128, NSB, SB)
        .transpose(1, 2, 0, 3).reshape(128, NHID * S))
    cosT = np.ascontiguousarray(cos.T).astype(BF16)
    sinT = np.ascontiguousarray(sin.T).astype(BF16)

    def wtile(wslice):
        # [HID, D] -> [128, NHID*D] with chunk t at columns [t*D, (t+1)*D)
        return np.ascontiguousarray(
            wslice.T.astype(BF16).reshape(NHID, 128, D)
            .transpose(1, 0, 2).reshape(128, NHID * D))
    wtq = (1.0 + np.asarray(q_norm_w, np.float32)).reshape(D, 1)
    wtk = (1.0 + np.asarray(k_norm_w, np.float32)).reshape(D, 1)

    i = np.arange(128)[:, None]
    j = np.arange(128)[None, :]
    mask = np.zeros((128, MAXW), np.float32)
    mask[:, 0:128] = np.where(j > i, 0.0, -2000.0)      # oldest chunk
    mask[:, 512:640] = np.where(j <= i, 0.0, -2000.0)   # causal chunk
    ident = np.eye(128, dtype=BF16)
    ones = np.ones((128, 1), BF16)

    in_maps = []
    for h in range(NCORES):
        kv = h // (H // KV)
        qs = slice(h * D, (h + 1) * D)
        ks = slice(kv * D, (kv + 1) * D)
        in_maps.append({
            "hsT": hsT,
            "wqT": wtile(Wq[qs, :]),
            "wkT": wtile(Wk[ks, :]),
            "wvT": wtile(Wv[ks, :]),
            "woT": np.ascontiguousarray(Wo[:, qs].T).astype(BF16),
            "cosT": cosT, "sinT": sinT,
            "wtq": wtq, "wtk": wtk,
            "maskadd": mask, "ident": ident, "ones_": ones,
        })
    return in_maps


def kernel(hidden_states, position_ids, cos_table, sin_table,
           Wq, Wk, Wv, Wo, q_norm_w, k_norm_w):
    global _COMPILED, LAST_RESULT
    trace = bool(os.environ.get("BASS_TRACE"))
    if trace:
        _install_ntff_shim()
    from concourse import bass_utils

    if _COMPILED is None:
        _COMPILED = _build()

    in_maps = _prep_in_maps(hidden_states, position_ids, cos_table,
                            sin_table, Wq, Wk, Wv, Wo, q_norm_w, k_norm_w)
    res = bass_utils.run_bass_kernel_spmd(
        _COMPILED, in_maps, core_ids=list(range(NCORES)), trace=trace)
    LAST_RESULT = res

    out = res.results[0]["out"].astype(np.float32)
    for i in range(1, NCORES):
        out += res.results[i]["out"]
    return out.reshape(B, S, HID)


# revision 33
# speedup vs baseline: 1.0360x; 1.0360x over previous
"""Gemma3-style sliding-window attention on 8 Trainium2 NeuronCores.

Sharding: tensor-parallel over the 8 query heads (1 head per core, KV head
h//2 replicated per pair). Each core computes its head's partial o-proj
output [S, HID]; the host sums the 8 partials.

All matmul operands are bf16 (rel-err budget 2e-2); accumulation is f32 in
PSUM. Everything the device consumes transposed is pre-transposed on the
host, so the device issues only natural-layout matmuls.

Softmax trick: scores are softcapped by 50*tanh(.), so they are bounded in
[-50, 50] and exp() never overflows f32 -> no running-max subtraction.
Masking is additive (-2000) on the tanh output (pre-exp), which makes
masked exp() terms exactly 0.
"""

import os
import sys
import types

import numpy as np
import ml_dtypes

BF16 = ml_dtypes.bfloat16

B, S, HID = 1, 2048, 2560
H, KV, D = 8, 4, 256
SCALE = 256 ** -0.5
SOFTCAP = 50.0
WINDOW = 512
EPS = 1e-6
NCORES = 8
DH = 2                 # 128-partition halves of D
NHID = HID // 128      # 20
SB = 512               # s-block size
NSB = S // SB          # 4
NBLK = S // 128        # 16 query blocks
MAXW = WINDOW + 128    # max key span per query block

_COMPILED = None
LAST_RESULT = None     # BassKernelResults of the most recent run (for test.py)


def _install_ntff_shim():
    """The image's antenv lacks axon_hooks; recreate it so trace=True works."""
    try:
        from antenv import axon_hooks  # noqa: F401
        return
    except ImportError:
        pass
    try:
        import antenv
        import trn_agent_boot.trn_boot as tb

        hook = tb._ntff_profile_via_ctypes("/opt/axon/libaxon_pjrt.so")
        mod = types.ModuleType("antenv.axon_hooks")
        mod._hook = hook
        mod.get_axon_ntff_profile_hook = lambda: mod._hook
        mod.set_axon_ntff_profile_hook = lambda h: setattr(mod, "_hook", h)
        sys.modules["antenv.axon_hooks"] = mod
        antenv.axon_hooks = mod
    except Exception:
        pass


def _build():
    import concourse.mybir as mybir
    import concourse.tile as tile
    from concourse import bacc
    from concourse.mybir import ActivationFunctionType as AF

    f32 = mybir.dt.float32
    bf16 = mybir.dt.bfloat16

    nc = bacc.Bacc("TRN2", target_bir_lowering=False, debug=False,
                   num_devices=NCORES)

    hsT_d = nc.dram_tensor("hsT", [128, NHID * S], bf16, kind="ExternalInput")
    wqT_d = nc.dram_tensor("wqT", [128, NHID * D], bf16, kind="ExternalInput")
    wkT_d = nc.dram_tensor("wkT", [128, NHID * D], bf16, kind="ExternalInput")
    wvT_d = nc.dram_tensor("wvT", [128, NHID * D], bf16, kind="ExternalInput")
    woT_d = nc.dram_tensor("woT", [D, HID], bf16, kind="ExternalInput")
    cosT_d = nc.dram_tensor("cosT", [D, S], bf16, kind="ExternalInput")
    sinT_d = nc.dram_tensor("sinT", [D, S], bf16, kind="ExternalInput")
    wtq_d = nc.dram_tensor("wtq", [D, 1], f32, kind="ExternalInput")
    wtk_d = nc.dram_tensor("wtk", [D, 1], f32, kind="ExternalInput")
    mask_d = nc.dram_tensor("maskadd", [128, MAXW], f32, kind="ExternalInput")
    id_d = nc.dram_tensor("ident", [128, 128], bf16, kind="ExternalInput")
    ones_d = nc.dram_tensor("ones_", [128, 1], bf16, kind="ExternalInput")
    out_d = nc.dram_tensor("out", [S, HID], bf16, kind="ExternalOutput")

    C2 = (SOFTCAP / SCALE) ** 2  # folds SCALE/SOFTCAP into the k-norm scale

    with tile.TileContext(nc) as tc:
        with tc.tile_pool(name="const", bufs=1) as cp, \
             tc.tile_pool(name="hstp", bufs=2) as hstp, \
             tc.tile_pool(name="evp", bufs=3) as evp, \
             tc.tile_pool(name="smp", bufs=2) as smp, \
             tc.tile_pool(name="psA", bufs=2, space="PSUM") as psA, \
             tc.tile_pool(name="psS1", bufs=2, space="PSUM") as psS1, \
             tc.tile_pool(name="psS2", bufs=1, space="PSUM") as psS2, \
             tc.tile_pool(name="psT", bufs=1, space="PSUM") as psT, \
             tc.tile_pool(name="psP", bufs=2, space="PSUM") as psP:

            # ---- persistent constants ----
            wq_sb = cp.tile([128, NHID * D], bf16, tag="wq", name="wq")
            wk_sb = cp.tile([128, NHID * D], bf16, tag="wk", name="wk")
            wv_sb = cp.tile([128, NHID * D], bf16, tag="wv", name="wv")
            wo_sb = [cp.tile([128, HID], bf16, tag=f"wo{d}", name=f"wo{d}")
                     for d in range(DH)]
            cos_sb = [cp.tile([128, S], bf16, tag=f"cos{d}", name=f"cos{d}")
                      for d in range(DH)]
            sin_sb = [cp.tile([128, S], bf16, tag=f"sin{d}", name=f"sin{d}")
                      for d in range(DH)]
            wtq_sb = [cp.tile([128, 1], f32, tag=f"wtq{d}", name=f"wtq{d}")
                      for d in range(DH)]
            wtk_sb = [cp.tile([128, 1], f32, tag=f"wtk{d}", name=f"wtk{d}")
                      for d in range(DH)]
            mask_sb = cp.tile([128, MAXW], f32, tag="mask", name="mask")
            id_sb = cp.tile([128, 128], bf16, tag="ident", name="ident")
            ones_sb = cp.tile([128, 1], bf16, tag="ones", name="ones")
            epsq_sb = cp.tile([128, 1], f32, tag="epsq", name="epsq")
            epsk_sb = cp.tile([128, 1], f32, tag="epsk", name="epsk")
            nc.vector.memset(epsq_sb, EPS)
            nc.vector.memset(epsk_sb, C2 * EPS)

            # persistent activations
            qwT = [cp.tile([128, S], bf16, tag=f"qwT{d}", name=f"qwT{d}")
                   for d in range(DH)]
            kwT = [cp.tile([128, S], bf16, tag=f"kwT{d}", name=f"kwT{d}")
                   for d in range(DH)]
            outT = [cp.tile([128, S], bf16, tag=f"outT{d}", name=f"outT{d}")
                    for d in range(DH)]
            v_sb = [cp.tile([128, D], bf16, tag=f"v{m}", name=f"v{m}")
                    for m in range(NBLK)]
            beta_bc = cp.tile([128, S], f32, tag="betabc", name="betabc")
            alpha = cp.tile([128, NBLK], f32, tag="alpha", name="alpha")
            dn = cp.tile([128, NBLK], f32, tag="dn", name="dn")
            rc = cp.tile([128, NBLK], f32, tag="rc", name="rc")

            # piecewise wide-line loads, interleaved so the first
            # projection matmuls start after the first piece lands
            SBW = NHID * SB      # columns per s-block in the streamed layout
            hst0 = hstp.tile([128, SBW], bf16, tag="hstbig", name="hst_sb0")
            NP = 4
            CP = NHID // NP      # weight chunks per piece
            for p in range(NP):
                wsl = slice(p * CP * D, (p + 1) * CP * D)
                nc.sync.dma_start(wq_sb[:, wsl], wqT_d.ap()[:, wsl])
                nc.sync.dma_start(wk_sb[:, wsl], wkT_d.ap()[:, wsl])
                hsl = slice(p * CP * SB, (p + 1) * CP * SB)
                nc.sync.dma_start(hst0[:, hsl], hsT_d.ap()[:, hsl])
            nc.sync.dma_start(wv_sb, wvT_d.ap())
            # needed from RoPE / softmax onward — after the first projection
            for d in range(DH):
                r = slice(d * 128, (d + 1) * 128)
                nc.sync.dma_start(cos_sb[d], cosT_d.ap()[r, :])
                nc.sync.dma_start(sin_sb[d], sinT_d.ap()[r, :])
                nc.sync.dma_start(wtq_sb[d], wtq_d.ap()[r, :])
                nc.sync.dma_start(wtk_sb[d], wtk_d.ap()[r, :])
            nc.sync.dma_start(mask_sb, mask_d.ap())
            nc.sync.dma_start(id_sb, id_d.ap())
            nc.sync.dma_start(ones_sb, ones_d.ap())
            # needed only at o-proj
            for d in range(DH):
                r = slice(d * 128, (d + 1) * 128)
                nc.sync.dma_start(wo_sb[d], woT_d.ap()[r, :])

            def emit_oproj_block(b):
                    qsl = slice(b * 128, (b + 1) * 128)
                    orow = smp.tile([128, HID], bf16, tag="orow", bufs=2,
                                    name=f"orow{b}")
                    for n in range(5):
                        op = psA.tile([128, SB], f32, tag="acc",
                                      name=f"op{b}_{n}")
                        nsl = slice(n * 512, (n + 1) * 512)
                        for d in range(DH):
                            nc.tensor.matmul(op, outT[d][:, qsl],
                                             wo_sb[d][:, nsl],
                                             start=(d == 0),
                                             stop=(d == DH - 1))
                        if n % 2 == 0:
                            nc.scalar.mul(orow[:, nsl], op, rc[:, b:b + 1])
                        else:
                            nc.vector.tensor_scalar_mul(orow[:, nsl], op,
                                                        rc[:, b:b + 1])
                    nc.sync.dma_start(out_d.ap()[b * 128:(b + 1) * 128, :],
                                      orow)

            hst_tiles = []
            for sb in range(NSB):
                s0 = sb * SB
                sl = slice(s0, s0 + SB)

                # ---- hidden-state stream for this s-block ----
                if sb == 0:
                    hst_tiles.append(hst0)
                if sb + 1 < NSB:
                    nxt = hstp.tile([128, SBW], bf16, tag="hstbig",
                                    name=f"hst_sb{sb + 1}")
                    nc.sync.dma_start(
                        nxt, hsT_d.ap()[:, (sb + 1) * SBW:(sb + 2) * SBW])
                    hst_tiles.append(nxt)
                hstb = hst_tiles[sb]

                # ---- q/k projections (transposed layout [d, s]) ----
                qsq = [evp.tile([128, SB], bf16, tag=f"qsq{d}",
                                name=f"qsq{sb}_{d}") for d in range(DH)]
                ksq = [evp.tile([128, SB], bf16, tag=f"ksq{d}",
                                name=f"ksq{sb}_{d}") for d in range(DH)]
                qtl = [evp.tile([128, SB], bf16, tag=f"qtl{d}",
                                name=f"qtl{sb}_{d}") for d in range(DH)]
                ktl = [evp.tile([128, SB], bf16, tag=f"ktl{d}",
                                name=f"ktl{sb}_{d}") for d in range(DH)]
                for wst, wtt, sqt, qt in ((wq_sb, wtq_sb, qsq, qtl),
                                          (wk_sb, wtk_sb, ksq, ktl)):
                    for d in range(DH):
                        pp = psA.tile([128, SB], f32, tag="acc",
                                      name=f"pp{sb}_{d}")
                        for t in range(NHID):
                            lsl = slice(t * D + d * 128,
                                        t * D + (d + 1) * 128)
                            nc.tensor.matmul(pp, wst[:, lsl],
                                             hstb[:, t * SB:(t + 1) * SB],
                                             start=(t == 0),
                                             stop=(t == NHID - 1))
                        nc.scalar.activation(sqt[d], pp, AF.Square)
                        nc.vector.tensor_scalar_mul(qt[d], pp, wtt[d])

                # ---- v projection (natural layout [s, d]) ----
                # emitted before RoPE: gives the PE matmul work while the
                # DVE runs the rope elementwise chain
                for m in range(4):
                    vp = psP.tile([128, D], f32, tag="pp",
                                   name=f"vp{sb}_{m}")
                    for t in range(NHID):
                        msl = slice(t * SB + m * 128, t * SB + (m + 1) * 128)
                        nc.tensor.matmul(vp, hstb[:, msl],
                                         wv_sb[:, t * D:(t + 1) * D],
                                         start=(t == 0), stop=(t == NHID - 1))
                    nc.vector.tensor_copy(v_sb[sb * 4 + m], vp)

                # ---- RoPE into persistent qwT/kwT (k first) ----
                for qt, dstT in ((ktl, kwT), (qtl, qwT)):
                    ra = smp.tile([128, SB], bf16, tag="ra", name=f"ra{sb}")
                    rb = smp.tile([128, SB], bf16, tag="rb", name=f"rb{sb}")
                    nc.vector.tensor_mul(ra, qt[0], cos_sb[0][:, sl])
                    nc.vector.tensor_mul(rb, qt[1], sin_sb[0][:, sl])
                    nc.vector.tensor_sub(dstT[0][:, sl], ra, rb)
                    rc2 = smp.tile([128, SB], bf16, tag="ra", name=f"rc{sb}")
                    rd = smp.tile([128, SB], bf16, tag="rb", name=f"rd{sb}")
                    nc.vector.tensor_mul(rc2, qt[1], cos_sb[1][:, sl])
                    nc.vector.tensor_mul(rd, qt[0], sin_sb[1][:, sl])
                    nc.vector.tensor_add(dstT[1][:, sl], rc2, rd)

    
                # previous s-block's o-proj: PE filler during rope/stats
                if sb > 0:
                    for mm in range(4):
                        emit_oproj_block((sb - 1) * 4 + mm)

                # ---- alpha = 1/sqrt(mean(q^2)+eps), per query row ----
                ap_ps = psP.tile([128, 4], f32, tag="pp",
                                 name=f"aps{sb}")
                for m in range(4):
                    msl = slice(m * 128, (m + 1) * 128)
                    for d in range(DH):
                        nc.tensor.matmul(ap_ps[:, m:m + 1], qsq[d][:, msl],
                                         ones_sb, start=(d == 0),
                                         stop=(d == DH - 1))
                atmp = smp.tile([128, 4], f32, tag="atmp", name=f"atmp{sb}")
                nc.scalar.activation(atmp, ap_ps, AF.Sqrt,
                                     bias=epsq_sb, scale=1.0 / D)
                nc.vector.reciprocal(alpha[:, sb * 4:(sb + 1) * 4], atmp)

                # ---- beta row = (SCALE/50)/sqrt(mean(k^2)+eps), bcast ----
                bp = psA.tile([1, SB], f32, tag="acc", name=f"bp{sb}")
                for d in range(DH):
                    nc.tensor.matmul(bp, ones_sb, ksq[d],
                                     start=(d == 0), stop=(d == DH - 1))
                btmp = smp.tile([1, SB], f32, tag="btmp", name=f"btmp{sb}")
                nc.scalar.activation(btmp, bp, AF.Sqrt,
                                     bias=epsk_sb[0:1, :], scale=C2 / D)
                brow = smp.tile([1, SB], f32, tag="brow", name=f"brow{sb}")
                nc.vector.reciprocal(brow, btmp)
                nc.gpsimd.partition_broadcast(beta_bc[:, sl], brow)

                # ---- attention + o-proj per 128-row query block ----
                for m in range(4):
                    b = sb * 4 + m
                    q0 = b * 128
                    w = min(b + 1, 5) * 128
                    k0 = q0 + 128 - w
                    w1 = w - 128
                    qsl = slice(q0, q0 + 128)

                    if w1 > 0:
                        sc1 = psS1.tile([128, SB], f32, tag="sc1",
                                        name=f"sc1_{b}")
                        for d in range(DH):
                            nc.tensor.matmul(sc1[:, 0:w1], qwT[d][:, qsl],
                                             kwT[d][:, k0:k0 + w1],
                                             start=(d == 0),
                                             stop=(d == DH - 1))
                    sc2 = psS2.tile([128, 128], f32, tag="sc2",
                                     name=f"sc2_{b}")
                    for d in range(DH):
                        nc.tensor.matmul(sc2, qwT[d][:, qsl], kwT[d][:, qsl],
                                         start=(d == 0), stop=(d == DH - 1))

                    traw = smp.tile([128, MAXW], f32, tag="traw", bufs=2,
                                    name=f"traw{b}")
                    if w1 > 0:
                        nc.vector.tensor_mul(traw[:, 0:w1], sc1[:, 0:w1],
                                             beta_bc[:, k0:k0 + w1])
                    nc.vector.tensor_mul(traw[:, w1:w], sc2, beta_bc[:, qsl])

                    tt = smp.tile([128, MAXW], f32, tag="tanh", bufs=2,
                                  name=f"tt{b}")
                    nc.scalar.activation(tt[:, 0:w], traw[:, 0:w], AF.Tanh,
                                         scale=alpha[:, b:b + 1])
                    nc.vector.tensor_add(tt[:, w1:w], tt[:, w1:w],
                                         mask_sb[:, 512:640])
                    if w == MAXW:
                        nc.vector.tensor_add(tt[:, 0:128], tt[:, 0:128],
                                             mask_sb[:, 0:128])

                    et = smp.tile([128, MAXW], bf16, tag="et", bufs=3,
                                  name=f"et{b}")
                    nc.scalar.activation(et[:, 0:w], tt[:, 0:w], AF.Exp,
                                         scale=SOFTCAP,
                                         accum_out=dn[:, b:b + 1])
                    # 1/denominator is applied at the o-proj eviction (rows
                    # of that PSUM are queries), keeping it off the softmax
                    # critical path
                    nc.vector.reciprocal(rc[:, b:b + 1], dn[:, b:b + 1])

                    nchunks = w // 128
                    etcs = []
                    for c in range(nchunks):
                        tp = psT.tile([128, 128], bf16, tag="tp",
                                      name=f"tp{b}_{c}")
                        nc.tensor.transpose(tp, et[:, c * 128:(c + 1) * 128],
                                            id_sb)
                        etc = smp.tile([128, 128], bf16, tag="etc", bufs=6,
                                       name=f"etc{b}_{c}")
                        nc.vector.tensor_copy(etc, tp)
                        etcs.append(etc)
                    po = psP.tile([128, D], f32, tag="pp", name=f"po{b}")
                    for d in range(DH):
                        dsl = slice(d * 128, (d + 1) * 128)
                        for c in range(nchunks):
                            kvi = k0 // 128 + c
                            nc.tensor.matmul(po[:, dsl], v_sb[kvi][:, dsl],
                                             etcs[c], start=(c == 0),
                                             stop=(c == nchunks - 1))
                    for d in range(DH):
                        dsl = slice(d * 128, (d + 1) * 128)
                        nc.vector.tensor_copy(outT[d][:, qsl], po[:, dsl])

                    if sb == NSB - 1:
                        emit_oproj_block(b)

    nc.compile()
    return nc


def _prep_in_maps(hidden_states, position_ids, cos_table, sin_table,
                  Wq, Wk, Wv, Wo, q_norm_w, k_norm_w):
    hs = np.asarray(hidden_states, np.float32).reshape(S, HID)
    pos = np.asarray(position_ids).reshape(S).astype(np.int64)
    cos = np.asarray(cos_table, np.float32)[pos]   # [S, D]
    sin = np.asarray(sin_table, np.float32)[pos]
    Wq = np.asarray(Wq, np.float32)
    Wk = np.asarray(Wk, np.float32)
    Wv = np.asarray(Wv, np.float32)
    Wo = np.asarray(Wo, np.float32)

    # streamed layout: [128, sb*(NHID*SB) + t*SB + s'] so every DMA line is
    # wide and contiguous
    hsT = np.ascontiguousarray(
        hs.T.astype(BF16).reshape(NHID, 128, NSB, SB)
        .transpose(1, 2, 0, 3).reshape(128, NHID * S))
    cosT = np.ascontiguousarray(cos.T).astype(BF16)
    sinT = np.ascontiguousarray(sin.T).astype(BF16)

    def wtile(wslice):
        # [HID, D] -> [128, NHID*D] with chunk t at columns [t*D, (t+1)*D)
        return np.ascontiguousarray(
            wslice.T.astype(BF16).reshape(NHID, 128, D)
            .transpose(1, 0, 2).reshape(128, NHID * D))
    wtq = (1.0 + np.asarray(q_norm_w, np.float32)).reshape(D, 1)
    wtk = (1.0 + np.asarray(k_norm_w, np.float32)).reshape(D, 1)

    i = np.arange(128)[:, None]
    j = np.arange(128)[None, :]
    mask = np.zeros((128, MAXW), np.float32)
    mask[:, 0:128] = np.where(j > i, 0.0, -2000.0)      # oldest chunk
    mask[:, 512:640] = np.where(j <= i, 0.0, -2000.0)   # causal chunk
    ident = np.eye(128, dtype=BF16)
    ones = np.ones((128, 1), BF16)

    in_maps = []
    for h in range(NCORES):
        kv = h // (H // KV)
        qs = slice(h * D, (h + 1) * D)
        ks = slice(kv * D, (kv + 1) * D)
        in_maps.append({
            "hsT": hsT,
            "wqT": wtile(Wq[qs, :]),
            "wkT": wtile(Wk[ks, :]),
            "wvT": wtile(Wv[ks, :]),
            "woT": np.ascontiguousarray(Wo[:, qs].T).astype(BF16),
            "cosT": cosT, "sinT": sinT,
            "wtq": wtq, "wtk": wtk,
            "maskadd": mask, "ident": ident, "ones_": ones,
        })
    return in_maps


def kernel(hidden_states, position_ids, cos_table, sin_table,
           Wq, Wk, Wv, Wo, q_norm_w, k_norm_w):
    global _COMPILED, LAST_RESULT
    trace = bool(os.environ.get("BASS_TRACE"))
    if trace:
        _install_ntff_shim()
    from concourse import bass_utils

    if _COMPILED is None:
        _COMPILED = _build()

    in_maps = _prep_in_maps(hidden_states, position_ids, cos_table,
                            sin_table, Wq, Wk, Wv, Wo, q_norm_w, k_norm_w)
    res = bass_utils.run_bass_kernel_spmd(
        _COMPILED, in_maps, core_ids=list(range(NCORES)), trace=trace)
    LAST_RESULT = res

    out = res.results[0]["out"].astype(np.float32)
    for i in range(1, NCORES):
        out += res.results[i]["out"]
    return out.reshape(B, S, HID)


# revision 35
# speedup vs baseline: 1.0611x; 1.0242x over previous
"""Gemma3-style sliding-window attention on 8 Trainium2 NeuronCores.

Sharding: tensor-parallel over the 8 query heads (1 head per core, KV head
h//2 replicated per pair). Each core computes its head's partial o-proj
output [S, HID]; the host sums the 8 partials.

All matmul operands are bf16 (rel-err budget 2e-2); accumulation is f32 in
PSUM. Everything the device consumes transposed is pre-transposed on the
host, so the device issues only natural-layout matmuls.

Softmax trick: scores are softcapped by 50*tanh(.), so they are bounded in
[-50, 50] and exp() never overflows f32 -> no running-max subtraction.
Masking is additive (-2000) on the tanh output (pre-exp), which makes
masked exp() terms exactly 0.
"""

import os
import sys
import types

import numpy as np
import ml_dtypes

BF16 = ml_dtypes.bfloat16

B, S, HID = 1, 2048, 2560
H, KV, D = 8, 4, 256
SCALE = 256 ** -0.5
SOFTCAP = 50.0
WINDOW = 512
EPS = 1e-6
NCORES = 8
DH = 2                 # 128-partition halves of D
NHID = HID // 128      # 20
SB = 512               # s-block size
NSB = S // SB          # 4
NBLK = S // 128        # 16 query blocks
MAXW = WINDOW + 128    # max key span per query block

_COMPILED = None
LAST_RESULT = None     # BassKernelResults of the most recent run (for test.py)


def _install_ntff_shim():
    """The image's antenv lacks axon_hooks; recreate it so trace=True works."""
    try:
        from antenv import axon_hooks  # noqa: F401
        return
    except ImportError:
        pass
    try:
        import antenv
        import trn_agent_boot.trn_boot as tb

        hook = tb._ntff_profile_via_ctypes("/opt/axon/libaxon_pjrt.so")
        mod = types.ModuleType("antenv.axon_hooks")
        mod._hook = hook
        mod.get_axon_ntff_profile_hook = lambda: mod._hook
        mod.set_axon_ntff_profile_hook = lambda h: setattr(mod, "_hook", h)
        sys.modules["antenv.axon_hooks"] = mod
        antenv.axon_hooks = mod
    except Exception:
        pass


def _build():
    import concourse.mybir as mybir
    import concourse.tile as tile
    from concourse import bacc
    from concourse.mybir import ActivationFunctionType as AF

    f32 = mybir.dt.float32
    bf16 = mybir.dt.bfloat16

    nc = bacc.Bacc("TRN2", target_bir_lowering=False, debug=False,
                   num_devices=NCORES)

    hsT_d = nc.dram_tensor("hsT", [128, NHID * S], bf16, kind="ExternalInput")
    wqT_d = nc.dram_tensor("wqT", [128, NHID * D], bf16, kind="ExternalInput")
    wkT_d = nc.dram_tensor("wkT", [128, NHID * D], bf16, kind="ExternalInput")
    wvT_d = nc.dram_tensor("wvT", [128, NHID * D], bf16, kind="ExternalInput")
    woT_d = nc.dram_tensor("woT", [D, HID], bf16, kind="ExternalInput")
    cosT_d = nc.dram_tensor("cosT", [D, S], bf16, kind="ExternalInput")
    sinT_d = nc.dram_tensor("sinT", [D, S], bf16, kind="ExternalInput")
    wtq_d = nc.dram_tensor("wtq", [D, 1], f32, kind="ExternalInput")
    wtk_d = nc.dram_tensor("wtk", [D, 1], f32, kind="ExternalInput")
    mask_d = nc.dram_tensor("maskadd", [128, MAXW], f32, kind="ExternalInput")
    id_d = nc.dram_tensor("ident", [128, 128], bf16, kind="ExternalInput")
    ones_d = nc.dram_tensor("ones_", [128, 1], bf16, kind="ExternalInput")
    out_d = nc.dram_tensor("out", [S, HID], bf16, kind="ExternalOutput")

    C2 = (SOFTCAP / SCALE) ** 2  # folds SCALE/SOFTCAP into the k-norm scale

    with tile.TileContext(nc) as tc:
        with tc.tile_pool(name="const", bufs=1) as cp, \
             tc.tile_pool(name="hstp", bufs=2) as hstp, \
             tc.tile_pool(name="evp", bufs=3) as evp, \
             tc.tile_pool(name="smp", bufs=2) as smp, \
             tc.tile_pool(name="psA", bufs=2, space="PSUM") as psA, \
             tc.tile_pool(name="psS1", bufs=2, space="PSUM") as psS1, \
             tc.tile_pool(name="psS2", bufs=1, space="PSUM") as psS2, \
             tc.tile_pool(name="psT", bufs=1, space="PSUM") as psT, \
             tc.tile_pool(name="psP", bufs=2, space="PSUM") as psP:

            # ---- persistent constants ----
            wq_sb = cp.tile([128, NHID * D], bf16, tag="wq", name="wq")
            wk_sb = cp.tile([128, NHID * D], bf16, tag="wk", name="wk")
            wv_sb = cp.tile([128, NHID * D], bf16, tag="wv", name="wv")
            wo_sb = [cp.tile([128, HID], bf16, tag=f"wo{d}", name=f"wo{d}")
                     for d in range(DH)]
            cos_sb = [cp.tile([128, S], bf16, tag=f"cos{d}", name=f"cos{d}")
                      for d in range(DH)]
            sin_sb = [cp.tile([128, S], bf16, tag=f"sin{d}", name=f"sin{d}")
                      for d in range(DH)]
            wtq_sb = [cp.tile([128, 1], f32, tag=f"wtq{d}", name=f"wtq{d}")
                      for d in range(DH)]
            wtk_sb = [cp.tile([128, 1], f32, tag=f"wtk{d}", name=f"wtk{d}")
                      for d in range(DH)]
            mask_sb = cp.tile([128, MAXW], f32, tag="mask", name="mask")
            id_sb = cp.tile([128, 128], bf16, tag="ident", name="ident")
            ones_sb = cp.tile([128, 1], bf16, tag="ones", name="ones")
            epsq_sb = cp.tile([128, 1], f32, tag="epsq", name="epsq")
            epsk_sb = cp.tile([128, 1], f32, tag="epsk", name="epsk")
            nc.vector.memset(epsq_sb, EPS)
            nc.vector.memset(epsk_sb, C2 * EPS)

            # persistent activations
            qwT = [cp.tile([128, S], bf16, tag=f"qwT{d}", name=f"qwT{d}")
                   for d in range(DH)]
            kwT = [cp.tile([128, S], bf16, tag=f"kwT{d}", name=f"kwT{d}")
                   for d in range(DH)]
            outT = [cp.tile([128, S], bf16, tag=f"outT{d}", name=f"outT{d}")
                    for d in range(DH)]
            v_sb = [cp.tile([128, D], bf16, tag=f"v{m}", name=f"v{m}")
                    for m in range(NBLK)]
            beta_bc = cp.tile([128, S], f32, tag="betabc", name="betabc")
            alpha = cp.tile([128, NBLK], f32, tag="alpha", name="alpha")
            dn = cp.tile([128, NBLK], f32, tag="dn", name="dn")
            rc = cp.tile([128, NBLK], f32, tag="rc", name="rc")

            # piecewise wide-line loads, interleaved so the first
            # projection matmuls start after the first piece lands
            SBW = NHID * SB      # columns per s-block in the streamed layout
            hst0 = hstp.tile([128, SBW], bf16, tag="hstbig", name="hst_sb0")
            NP = 4
            CP = NHID // NP      # weight chunks per piece
            for p in range(NP):
                wsl = slice(p * CP * D, (p + 1) * CP * D)
                nc.sync.dma_start(wq_sb[:, wsl], wqT_d.ap()[:, wsl])
                nc.sync.dma_start(wk_sb[:, wsl], wkT_d.ap()[:, wsl])
                hsl = slice(p * CP * SB, (p + 1) * CP * SB)
                nc.sync.dma_start(hst0[:, hsl], hsT_d.ap()[:, hsl])
            nc.sync.dma_start(wv_sb, wvT_d.ap())
            # needed from RoPE / softmax onward — after the first projection
            for d in range(DH):
                r = slice(d * 128, (d + 1) * 128)
                nc.sync.dma_start(cos_sb[d], cosT_d.ap()[r, :])
                nc.sync.dma_start(sin_sb[d], sinT_d.ap()[r, :])
                nc.sync.dma_start(wtq_sb[d], wtq_d.ap()[r, :])
                nc.sync.dma_start(wtk_sb[d], wtk_d.ap()[r, :])
            nc.sync.dma_start(mask_sb, mask_d.ap())
            nc.sync.dma_start(id_sb, id_d.ap())
            nc.sync.dma_start(ones_sb, ones_d.ap())
            # needed only at o-proj
            for d in range(DH):
                r = slice(d * 128, (d + 1) * 128)
                nc.sync.dma_start(wo_sb[d], woT_d.ap()[r, :])

            def emit_oproj_block(b):
                    qsl = slice(b * 128, (b + 1) * 128)
                    orow = smp.tile([128, HID], bf16, tag="orow", bufs=3,
                                    name=f"orow{b}")
                    for n in range(5):
                        op = psA.tile([128, SB], f32, tag="acc",
                                      name=f"op{b}_{n}")
                        nsl = slice(n * 512, (n + 1) * 512)
                        for d in range(DH):
                            nc.tensor.matmul(op, outT[d][:, qsl],
                                             wo_sb[d][:, nsl],
                                             start=(d == 0),
                                             stop=(d == DH - 1))
                        if n % 2 == 0:
                            nc.scalar.mul(orow[:, nsl], op, rc[:, b:b + 1])
                        else:
                            nc.vector.tensor_scalar_mul(orow[:, nsl], op,
                                                        rc[:, b:b + 1])
                    nc.sync.dma_start(out_d.ap()[b * 128:(b + 1) * 128, :],
                                      orow)

            hst_tiles = []
            for sb in range(NSB):
                s0 = sb * SB
                sl = slice(s0, s0 + SB)

                # ---- hidden-state stream for this s-block ----
                if sb == 0:
                    hst_tiles.append(hst0)
                if sb + 1 < NSB:
                    nxt = hstp.tile([128, SBW], bf16, tag="hstbig",
                                    name=f"hst_sb{sb + 1}")
                    nc.sync.dma_start(
                        nxt, hsT_d.ap()[:, (sb + 1) * SBW:(sb + 2) * SBW])
                    hst_tiles.append(nxt)
                hstb = hst_tiles[sb]

                # ---- q/k projections (transposed layout [d, s]) ----
                qsq = [evp.tile([128, SB], bf16, tag=f"qsq{d}",
                                name=f"qsq{sb}_{d}") for d in range(DH)]
                ksq = [evp.tile([128, SB], bf16, tag=f"ksq{d}",
                                name=f"ksq{sb}_{d}") for d in range(DH)]
                qtl = [evp.tile([128, SB], bf16, tag=f"qtl{d}",
                                name=f"qtl{sb}_{d}") for d in range(DH)]
                ktl = [evp.tile([128, SB], bf16, tag=f"ktl{d}",
                                name=f"ktl{sb}_{d}") for d in range(DH)]
                for wst, wtt, sqt, qt in ((wq_sb, wtq_sb, qsq, qtl),
                                          (wk_sb, wtk_sb, ksq, ktl)):
                    for d in range(DH):
                        pp = psA.tile([128, SB], f32, tag="acc",
                                      name=f"pp{sb}_{d}")
                        for t in range(NHID):
                            lsl = slice(t * D + d * 128,
                                        t * D + (d + 1) * 128)
                            nc.tensor.matmul(pp, wst[:, lsl],
                                             hstb[:, t * SB:(t + 1) * SB],
                                             start=(t == 0),
                                             stop=(t == NHID - 1))
                        nc.scalar.activation(sqt[d], pp, AF.Square)
                        nc.vector.tensor_scalar_mul(qt[d], pp, wtt[d])

                # ---- v projection (natural layout [s, d]) ----
                # emitted before RoPE: gives the PE matmul work while the
                # DVE runs the rope elementwise chain
                for m in range(4):
                    vp = psP.tile([128, D], f32, tag="pp",
                                   name=f"vp{sb}_{m}")
                    for t in range(NHID):
                        msl = slice(t * SB + m * 128, t * SB + (m + 1) * 128)
                        nc.tensor.matmul(vp, hstb[:, msl],
                                         wv_sb[:, t * D:(t + 1) * D],
                                         start=(t == 0), stop=(t == NHID - 1))
                    nc.vector.tensor_copy(v_sb[sb * 4 + m], vp)

                # ---- RoPE into persistent qwT/kwT (k first) ----
                for qt, dstT in ((ktl, kwT), (qtl, qwT)):
                    ra = smp.tile([128, SB], bf16, tag="ra", name=f"ra{sb}")
                    rb = smp.tile([128, SB], bf16, tag="rb", name=f"rb{sb}")
                    nc.vector.tensor_mul(ra, qt[0], cos_sb[0][:, sl])
                    nc.vector.tensor_mul(rb, qt[1], sin_sb[0][:, sl])
                    nc.vector.tensor_sub(dstT[0][:, sl], ra, rb)
                    rc2 = smp.tile([128, SB], bf16, tag="ra", name=f"rc{sb}")
                    rd = smp.tile([128, SB], bf16, tag="rb", name=f"rd{sb}")
                    nc.vector.tensor_mul(rc2, qt[1], cos_sb[1][:, sl])
                    nc.vector.tensor_mul(rd, qt[0], sin_sb[1][:, sl])
                    nc.vector.tensor_add(dstT[1][:, sl], rc2, rd)

    
                # previous s-block's o-proj: PE filler during rope/stats
                if sb > 0:
                    for mm in range(4):
                        emit_oproj_block((sb - 1) * 4 + mm)

                # ---- alpha = 1/sqrt(mean(q^2)+eps), per query row ----
                ap_ps = psP.tile([128, 4], f32, tag="pp",
                                 name=f"aps{sb}")
                for m in range(4):
                    msl = slice(m * 128, (m + 1) * 128)
                    for d in range(DH):
                        nc.tensor.matmul(ap_ps[:, m:m + 1], qsq[d][:, msl],
                                         ones_sb, start=(d == 0),
                                         stop=(d == DH - 1))
                atmp = smp.tile([128, 4], f32, tag="atmp", name=f"atmp{sb}")
                nc.scalar.activation(atmp, ap_ps, AF.Sqrt,
                                     bias=epsq_sb, scale=1.0 / D)
                nc.vector.reciprocal(alpha[:, sb * 4:(sb + 1) * 4], atmp)

                # ---- beta row = (SCALE/50)/sqrt(mean(k^2)+eps), bcast ----
                bp = psA.tile([1, SB], f32, tag="acc", name=f"bp{sb}")
                for d in range(DH):
                    nc.tensor.matmul(bp, ones_sb, ksq[d],
                                     start=(d == 0), stop=(d == DH - 1))
                btmp = smp.tile([1, SB], f32, tag="btmp", name=f"btmp{sb}")
                nc.scalar.activation(btmp, bp, AF.Sqrt,
                                     bias=epsk_sb[0:1, :], scale=C2 / D)
                brow = smp.tile([1, SB], f32, tag="brow", name=f"brow{sb}")
                nc.vector.reciprocal(brow, btmp)
                nc.gpsimd.partition_broadcast(beta_bc[:, sl], brow)

                # ---- attention + o-proj per 128-row query block ----
                for m in range(4):
                    b = sb * 4 + m
                    q0 = b * 128
                    w = min(b + 1, 5) * 128
                    k0 = q0 + 128 - w
                    w1 = w - 128
                    qsl = slice(q0, q0 + 128)

                    if w1 > 0:
                        sc1 = psS1.tile([128, SB], f32, tag="sc1",
                                        name=f"sc1_{b}")
                        for d in range(DH):
                            nc.tensor.matmul(sc1[:, 0:w1], qwT[d][:, qsl],
                                             kwT[d][:, k0:k0 + w1],
                                             start=(d == 0),
                                             stop=(d == DH - 1))
                    sc2 = psS2.tile([128, 128], f32, tag="sc2",
                                     name=f"sc2_{b}")
                    for d in range(DH):
                        nc.tensor.matmul(sc2, qwT[d][:, qsl], kwT[d][:, qsl],
                                         start=(d == 0), stop=(d == DH - 1))

                    traw = smp.tile([128, MAXW], f32, tag="traw", bufs=2,
                                    name=f"traw{b}")
                    if w1 > 0:
                        nc.vector.tensor_mul(traw[:, 0:w1], sc1[:, 0:w1],
                                             beta_bc[:, k0:k0 + w1])
                    nc.vector.tensor_mul(traw[:, w1:w], sc2, beta_bc[:, qsl])

                    tt = smp.tile([128, MAXW], f32, tag="tanh", bufs=2,
                                  name=f"tt{b}")
                    nc.scalar.activation(tt[:, 0:w], traw[:, 0:w], AF.Tanh,
                                         scale=alpha[:, b:b + 1])
                    nc.vector.tensor_add(tt[:, w1:w], tt[:, w1:w],
                                         mask_sb[:, 512:640])
                    if w == MAXW:
                        nc.vector.tensor_add(tt[:, 0:128], tt[:, 0:128],
                                             mask_sb[:, 0:128])

                    et = smp.tile([128, MAXW], bf16, tag="et", bufs=4,
                                  name=f"et{b}")
                    nc.scalar.activation(et[:, 0:w], tt[:, 0:w], AF.Exp,
                                         scale=SOFTCAP,
                                         accum_out=dn[:, b:b + 1])
                    # 1/denominator is applied at the o-proj eviction (rows
                    # of that PSUM are queries), keeping it off the softmax
                    # critical path
                    nc.vector.reciprocal(rc[:, b:b + 1], dn[:, b:b + 1])

                    nchunks = w // 128
                    etcs = []
                    for c in range(nchunks):
                        tp = psT.tile([128, 128], bf16, tag="tp",
                                      name=f"tp{b}_{c}")
                        nc.tensor.transpose(tp, et[:, c * 128:(c + 1) * 128],
                                            id_sb)
                        etc = smp.tile([128, 128], bf16, tag="etc", bufs=6,
                                       name=f"etc{b}_{c}")
                        nc.vector.tensor_copy(etc, tp)
                        etcs.append(etc)
                    po = psP.tile([128, D], f32, tag="pp", name=f"po{b}")
                    for d in range(DH):
                        dsl = slice(d * 128, (d + 1) * 128)
                        for c in range(nchunks):
                            kvi = k0 // 128 + c
                            nc.tensor.matmul(po[:, dsl], v_sb[kvi][:, dsl],
                                             etcs[c], start=(c == 0),
                                             stop=(c == nchunks - 1))
                    for d in range(DH):
                        dsl = slice(d * 128, (d + 1) * 128)
                        nc.vector.tensor_copy(outT[d][:, qsl], po[:, dsl])

                    if sb == NSB - 1:
                        emit_oproj_block(b)

    nc.compile()
    return nc


def _prep_in_maps(hidden_states, position_ids, cos_table, sin_table,
                  Wq, Wk, Wv, Wo, q_norm_w, k_norm_w):
    hs = np.asarray(hidden_states, np.float32).reshape(S, HID)
    pos = np.asarray(position_ids).reshape(S).astype(np.int64)
    cos = np.asarray(cos_table, np.float32)[pos]   # [S, D]
    sin = np.asarray(sin_table, np.float32)[pos]
    Wq = np.asarray(Wq, np.float32)
    Wk = np.asarray(Wk, np.float32)
    Wv = np.asarray(Wv, np.float32)
    Wo = np.asarray(Wo, np.float32)

    # streamed layout: [128, sb*(NHID*SB) + t*SB + s'] so every DMA line is
    # wide and contiguous
    hsT = np.ascontiguousarray(
        hs.T.astype(BF16).reshape(NHID, 128, NSB, SB)
        .transpose(1, 2, 0, 3).reshape(128, NHID * S))
    cosT = np.ascontiguousarray(cos.T).astype(BF16)
    sinT = np.ascontiguousarray(sin.T).astype(BF16)

    def wtile(wslice):
        # [HID, D] -> [128, NHID*D] with chunk t at columns [t*D, (t+1)*D)
        return np.ascontiguousarray(
            wslice.T.astype(BF16).reshape(NHID, 128, D)
            .transpose(1, 0, 2).reshape(128, NHID * D))
    wtq = (1.0 + np.asarray(q_norm_w, np.float32)).reshape(D, 1)
    wtk = (1.0 + np.asarray(k_norm_w, np.float32)).reshape(D, 1)

    i = np.arange(128)[:, None]
    j = np.arange(128)[None, :]
    mask = np.zeros((128, MAXW), np.float32)
    mask[:, 0:128] = np.where(j > i, 0.0, -2000.0)      # oldest chunk
    mask[:, 512:640] = np.where(j <= i, 0.0, -2000.0)   # causal chunk
    ident = np.eye(128, dtype=BF16)
    ones = np.ones((128, 1), BF16)

    in_maps = []
    for h in range(NCORES):
        kv = h // (H // KV)
        qs = slice(h * D, (h + 1) * D)
        ks = slice(kv * D, (kv + 1) * D)
        in_maps.append({
            "hsT": hsT,
            "wqT": wtile(Wq[qs, :]),
            "wkT": wtile(Wk[ks, :]),
            "wvT": wtile(Wv[ks, :]),
            "woT": np.ascontiguousarray(Wo[:, qs].T).astype(BF16),
            "cosT": cosT, "sinT": sinT,
            "wtq": wtq, "wtk": wtk,
            "maskadd": mask, "ident": ident, "ones_": ones,
        })
    return in_maps


def kernel(hidden_states, position_ids, cos_table, sin_table,
           Wq, Wk, Wv, Wo, q_norm_w, k_norm_w):
    global _COMPILED, LAST_RESULT
    trace = bool(os.environ.get("BASS_TRACE"))
    if trace:
        _install_ntff_shim()
    from concourse import bass_utils

    if _COMPILED is None:
        _COMPILED = _build()

    in_maps = _prep_in_maps(hidden_states, position_ids, cos_table,
                            sin_table, Wq, Wk, Wv, Wo, q_norm_w, k_norm_w)
    res = bass_utils.run_bass_kernel_spmd(
        _COMPILED, in_maps, core_ids=list(range(NCORES)), trace=trace)
    LAST_RESULT = res

    out = res.results[0]["out"].astype(np.float32)
    for i in range(1, NCORES):
        out += res.results[i]["out"]
    return out.reshape(B, S, HID)
